# revision 6
# baseline (speedup 1.0000x reference)
"""Trainium2 Bass kernel for nn_PlasticityModelMoE (8-core SPMD).

Strategy (v2, fp8):
  Host precomputes all weight transforms: wmod = w*sigmoid(delay)*conn*mask*64
  (b-major, fp8e4m3) concatenated with gate_W*64; x pre-transposed to fp8 xT;
  read_W shard scaled *64 fp8; memory shard (+ones col) fp16; connectivity MLP
  and activation-blend polynomial coefs (scaled *16) computed on host.
  Device: phase 1 (units tensor-parallel, 256/core): branch+gate logits via
  fp8 DoubleRow matmuls (K=256/step), gate softmax, z-combine, relu, degree-4
  Horner blend -> fp8 blendT (*16).  Per 512-col batch chunk: fp8 AllGather of
  blendT, then phase 3 (memory-rows tensor-parallel, 1024/core): logitsT =
  rw8 x bT via fp8 DoubleRow, exp (descale 2^-10, +read_b) -> fp16 expT.
  Phase 4: [read_partial | s] = E @ [mem | 1] in fp16, fp16 ReduceScatter over
  batch rows, divide by s -> each core emits its 256-row f32 output shard.
  A dummy 64B AllGather at t=0 absorbs the first-collective rendezvous.
"""
import numpy as np
from contextlib import ExitStack

import concourse.bass as bass
import concourse.mybir as mybir
import concourse.tile as tile
from concourse import bacc
from concourse.bass_utils import run_bass_kernel_spmd
from concourse.masks import make_identity

F32 = mybir.dt.float32
BF16 = mybir.dt.bfloat16
F16 = mybir.dt.float16
F8 = mybir.dt.float8e4
AF = mybir.ActivationFunctionType
ALU = mybir.AluOpType
AX = mybir.AxisListType
PM = mybir.MatmulPerfMode

KC = 8
N, D, U, NB, M, MD = 2048, 1024, 2048, 4, 8192, 1024
US = U // KC          # 256 units per core
MS = M // KC          # 1024 memory rows per core
NS = N // KC          # 256 output rows per core
NT = N // 128         # 16 batch tiles
DK = D // 128         # 8 k-tiles over D
DR = DK // 2          # 4 DoubleRow steps over D
UK = U // 128         # 16 k-tiles over U
UR = UK // 2          # 8 DoubleRow steps over U
MK = MS // 128        # 8 k-tiles over memory shard
UBF = US * NB         # 1024 branch columns per core
CH, CW = 4, 512       # batch chunks for collectives
SC_W = 64.0           # fp8 weight scale (2^6)
SC_B = 16.0           # fp8 blend scale (2^4)
DESC = 1.0 / (SC_W * SC_B)

_CMAT = np.array([
    [5.0000238e-01, 2.4987496e-01, 1.0582031e-03, -2.4046743e-02, 4.1678566e-03],
    [0.0, 1.0, 0.0, 0.0, 0.0],
    [-7.2632770e-06, 9.9976927e-01, 9.2018498e-03, -3.9401752e-01, 1.4669961e-01],
    [0.0, 1.0, 0.0, 0.0, 0.0],
    [8.6798245e-06, 4.9957812e-01, 2.5321743e-01, -8.1970906e-03, -1.3558048e-02],
    [3.9388153e-05, 4.9807969e-01, 4.1364601e-01, -3.7666172e-02, -3.2796454e-02],
    [0.0, 1.0507009873554805, 0.0, 0.0, 0.0],
    [3.1482985e-05, 5.9846270e-01, 3.3178753e-01, -4.6201140e-02, -1.9015398e-02],
    [0.0, 0.0, 0.0, 0.0, 0.0],
], dtype=np.float64)

_cache = {}


def _build(has_bias):
    nc = bacc.Bacc(num_devices=KC)

    xt_d = nc.dram_tensor("xt", [D, N], F8, kind="ExternalInput")
    wd_d = nc.dram_tensor("wd", [D, UBF + NB], F8, kind="ExternalInput")
    bias_d = nc.dram_tensor("bias", [UBF + NB], BF16, kind="ExternalInput")
    coefs_d = nc.dram_tensor("coefs", [128, 5], F32, kind="ExternalInput")
    rw_d = nc.dram_tensor("rw", [U, MS], F8, kind="ExternalInput")
    rb_d = nc.dram_tensor("rb", [MS], F32, kind="ExternalInput")
    mem_d = nc.dram_tensor("mem", [MS, MD + 1], F16, kind="ExternalInput")
    y_d = nc.dram_tensor("y", [NS, MD], F32, kind="ExternalOutput")

    with tile.TileContext(nc) as tc, ExitStack() as ctx:
        consts = ctx.enter_context(tc.tile_pool(name="consts", bufs=1))
        big = ctx.enter_context(tc.tile_pool(name="big", bufs=1))
        p3p = ctx.enter_context(tc.tile_pool(name="p3p", bufs=2))
        p4p = ctx.enter_context(tc.tile_pool(name="p4p", bufs=2))
        blendp = ctx.enter_context(tc.tile_pool(name="blendp", bufs=2))
        dram_ag = ctx.enter_context(tc.tile_pool(name="dram_ag", bufs=1, space="DRAM"))
        dram_rs = ctx.enter_context(tc.tile_pool(name="dram_rs", bufs=1, space="DRAM"))
        # PSUM budget (8 banks): br [128,1028] f32 ~2 banks x 2 bufs,
        # tr [128,<=512] 1 bank x 2 bufs
        psum = ctx.enter_context(tc.tile_pool(name="psum", bufs=2, space="PSUM"))

        # ---------- tiny consts ----------
        idf = consts.tile([128, 128], F32)
        make_identity(nc, idf)
        idb = consts.tile([128, 128], BF16)
        nc.any.tensor_copy(idb, idf)
        ones_lhs = consts.tile([1, 128], BF16)
        nc.vector.memset(ones_lhs, 1.0)
        bias_b = consts.tile([1, UBF + NB], BF16)
        nc.sync.dma_start(out=bias_b, in_=bias_d.ap()[None])
        coefs = consts.tile([128, 5], F32)
        nc.sync.dma_start(out=coefs, in_=coefs_d[:, :])
        rb_sb = consts.tile([128, MK], F32)
        nc.sync.dma_start(out=rb_sb, in_=rb_d.ap().rearrange("(t p) -> p t", p=128))

        # ---------- dummy collective to absorb first-cc rendezvous ----------
        dummy_sb = consts.tile([1, 16], F32)
        nc.vector.memset(dummy_sb, 0.0)
        dummy_in = dram_ag.tile([1, 16], F32, name="dummy_in", tag="dmi")
        nc.gpsimd.dma_start(out=dummy_in, in_=dummy_sb)
        dummy_out = dram_ag.tile([KC, 16], F32, name="dummy_out", tag="dmo",
                                 addr_space="Shared")
        nc.gpsimd.collective_compute(
            "AllGather", ALU.bypass, replica_groups=[list(range(KC))],
            ins=[dummy_in.opt()], outs=[dummy_out.opt()])

        # ---------- big input loads (sync queue, priority order) ----------
        wm = big.tile([128, DK, UBF + NB], F8)
        nc.sync.dma_start(out=wm, in_=wd_d.ap().rearrange("(t p) c -> p t c", p=128))
        xT = big.tile([128, DK, N], F8)
        for c in range(CH):
            csl = slice(c * CW, (c + 1) * CW)
            nc.sync.dma_start(
                out=xT[:, :, csl],
                in_=xt_d.ap()[:, csl].rearrange("(t p) n -> p t n", p=128))
        rw8 = big.tile([128, UK, MS], F8)
        nc.sync.dma_start(out=rw8, in_=rw_d.ap().rearrange("(t p) m -> p t m", p=128))
        mem16 = big.tile([128, MK, MD + 1], F16)
        nc.sync.dma_start(out=mem16, in_=mem_d.ap().rearrange("(t p) c -> p t c", p=128))

        blendT = big.tile([128, 2, N], F8)
        ag_outs = []
        expTs = []
        rs_outs = []
        blend16s = {}

        def emit_tile(i):
            nsl = slice(i * 128, (i + 1) * 128)
            br = psum.tile([128, UBF + NB], F32, tag="br", name="br")
            for (c0, c1) in [(0, 512), (512, 1024), (1024, 1028)]:
                for s in range(DR):
                    nc.tensor.matmul(br[:, c0:c1],
                                     xT[:, 2 * s:2 * s + 2, nsl],
                                     wm[:, 2 * s:2 * s + 2, c0:c1],
                                     start=(s == 0),
                                     stop=(not has_bias and s == DR - 1),
                                     perf_mode=PM.DoubleRow)
                if has_bias:
                    nc.tensor.matmul(br[:, c0:c1], ones_lhs, bias_b[:, c0:c1],
                                     start=False, stop=True,
                                     skip_group_check=True)
            # gate softmax on br[:, 1024:1028] (logits are *SC_W; exp safe
            # without max-sub: true |logit| <~ 4)
            g_exp = blendp.tile([128, NB], F32, tag="g1")
            nc.scalar.activation(g_exp, br[:, UBF:UBF + NB], AF.Exp,
                                 scale=1.0 / SC_W)
            g_sum = blendp.tile([128, 1], F32, tag="g2")
            nc.vector.tensor_reduce(g_sum, g_exp, AX.X, ALU.add)
            g_rec = blendp.tile([128, 1], F32, tag="g3")
            nc.vector.reciprocal(g_rec, g_sum)
            gate = blendp.tile([128, NB], F32, tag="g4")
            nc.any.tensor_scalar(gate, g_exp, g_rec[:, 0:1], 1.0 / SC_W,
                                 ALU.mult, ALU.mult)
            # z = sum_b gate_b * branch_b  (bf16 pipeline)
            zt0 = blendp.tile([128, US], BF16, tag="t0")
            nc.any.tensor_scalar_mul(zt0, br[:, 0:US], gate[:, 0:1])
            zt1 = blendp.tile([128, US], BF16, tag="t1")
            nc.any.tensor_scalar_mul(zt1, br[:, US:2 * US], gate[:, 1:2])
            zt2 = blendp.tile([128, US], BF16, tag="t2")
            nc.any.tensor_scalar_mul(zt2, br[:, 2 * US:3 * US], gate[:, 2:3])
            zt3 = blendp.tile([128, US], BF16, tag="t3")
            nc.any.tensor_scalar_mul(zt3, br[:, 3 * US:4 * US], gate[:, 3:4])
            z01 = blendp.tile([128, US], BF16, tag="t0")
            nc.any.tensor_add(z01, zt0, zt1)
            z23 = blendp.tile([128, US], BF16, tag="t2")
            nc.any.tensor_add(z23, zt2, zt3)
            z_sb = blendp.tile([128, US], BF16, tag="t1")
            nc.any.tensor_add(z_sb, z01, z23)
            a_sb = blendp.tile([128, US], BF16, tag="ta")
            nc.any.tensor_scalar_max(a_sb, z_sb, 0.0)
            # blend*16 via degree-4 Horner (coefs prescaled *16)
            hp = blendp.tile([128, US], BF16, tag="t0")
            nc.any.tensor_scalar(hp, a_sb, coefs[:, 4:5], coefs[:, 3:4],
                                 ALU.mult, ALU.add)
            hq = blendp.tile([128, US], BF16, tag="t2")
            nc.any.tensor_mul(hq, hp, a_sb)
            hr = blendp.tile([128, US], BF16, tag="t0")
            nc.any.tensor_scalar_add(hr, hq, coefs[:, 2:3])
            hs = blendp.tile([128, US], BF16, tag="t2")
            nc.any.tensor_mul(hs, hr, a_sb)
            ht = blendp.tile([128, US], BF16, tag="t0")
            nc.any.tensor_scalar_add(ht, hs, coefs[:, 1:2])
            hu = blendp.tile([128, US], BF16, tag="t2")
            nc.any.tensor_mul(hu, ht, a_sb)
            blend16 = blendp.tile([128, US], BF16, tag="bb", bufs=5)
            nc.any.tensor_scalar_add(blend16, hu, coefs[:, 0:1])
            blend16s[i] = blend16

        def emit_transpose(i):
            nsl = slice(i * 128, (i + 1) * 128)
            blend16 = blend16s.pop(i)
            for uh in range(2):
                trb = psum.tile([128, 128], BF16, tag="tr", name="trb")
                nc.tensor.transpose(trb, blend16[:, uh * 128:(uh + 1) * 128], idb)
                nc.any.tensor_copy(blendT[:, uh, nsl], trb)

        def emit_ag(ch):
            csl = slice(ch * CW, (ch + 1) * CW)
            agi = dram_ag.tile([US, CW], F8, name=f"ag_in{ch}", tag=f"agi{ch}")
            for uh in range(2):
                nc.gpsimd.dma_start(out=agi[uh * 128:(uh + 1) * 128, :],
                                    in_=blendT[:, uh, csl])
            ago = dram_ag.tile([U, CW], F8, name=f"ag_out{ch}", tag=f"ago{ch}",
                               addr_space="Shared")
            nc.gpsimd.collective_compute(
                "AllGather", ALU.bypass, replica_groups=[list(range(KC))],
                ins=[agi.opt()], outs=[ago.opt()])
            ag_outs.append(ago)

        def emit_phase3(ch):
            bT = p3p.tile([128, UK, CW], F8, tag="bT", name="bT")
            for t in range(UK):
                nc.sync.dma_start(out=bT[:, t, :],
                                  in_=ag_outs[ch][t * 128:(t + 1) * 128, :])
            expT = p3p.tile([128, MK, CW], F16, tag="expT", name="expT")
            for mk in range(MK):
                l_ps = psum.tile([128, CW], F32, tag="tr", name="l_ps")
                for s in range(UR):
                    nc.tensor.matmul(l_ps,
                                     rw8[:, 2 * s:2 * s + 2,
                                         mk * 128:(mk + 1) * 128],
                                     bT[:, 2 * s:2 * s + 2, :],
                                     start=(s == 0), stop=(s == UR - 1),
                                     perf_mode=PM.DoubleRow)
                nc.scalar.activation(expT[:, mk, :], l_ps, AF.Exp,
                                     bias=rb_sb[:, mk:mk + 1], scale=DESC)
            expTs.append(expT)

        def emit_phase4(ch):
            expT = expTs[ch]
            for sj in range(4):
                jsl = slice(sj * 128, (sj + 1) * 128)
                r_ps = psum.tile([128, UBF + NB], F32, tag="br", name="r_ps")
                for (c0, c1) in [(0, 512), (512, 1024), (1024, 1025)]:
                    for mk in range(MK):
                        nc.tensor.matmul(r_ps[:, c0:c1], expT[:, mk, jsl],
                                         mem16[:, mk, c0:c1],
                                         start=(mk == 0), stop=(mk == MK - 1))
                r_sb = p4p.tile([128, MD + 1], F16, tag="rsb")
                nc.any.tensor_copy(r_sb, r_ps[:, 0:MD + 1])
                rs_inj = dram_rs.tile([128, MD + 1], F16,
                                      name=f"rs_in{ch}_{sj}", tag=f"rsi{ch}{sj}")
                nc.gpsimd.dma_start(out=rs_inj, in_=r_sb)
                rs_out = dram_rs.tile([16, MD + 1], F16,
                                      name=f"rs_out{ch}_{sj}", tag=f"rso{ch}{sj}")
                nc.gpsimd.collective_compute(
                    "ReduceScatter", ALU.add, replica_groups=[list(range(KC))],
                    ins=[rs_inj.opt()], outs=[rs_out.opt()])
                rs_outs.append(rs_out)

        def emit_epilogue(ch):
            for sj in range(4):
                e_f = p4p.tile([16, MD + 1], F16, tag="ef", name="e_f")
                nc.scalar.dma_start(out=e_f, in_=rs_outs[ch * 4 + sj][:, :])
                s_rec = p4p.tile([16, 1], F32, tag="sr", name="s_rec")
                nc.vector.reciprocal(s_rec, e_f[:, MD:MD + 1])
                y_t = p4p.tile([16, MD], F32, tag="yt", name="y_t")
                nc.any.tensor_scalar_mul(y_t, e_f[:, 0:MD], s_rec[:, 0:1])
                nc.gpsimd.dma_start(
                    out=y_d[ch * 64 + sj * 16:ch * 64 + sj * 16 + 16, :],
                    in_=y_t)

        # ---------- pipelined emission ----------
        for ch in range(CH):
            for it in range(4):
                emit_tile(ch * 4 + it)
            for it in range(4):
                emit_transpose(ch * 4 + it)
            emit_ag(ch)
            if ch >= 1:
                emit_phase3(ch - 1)
                emit_phase4(ch - 1)
            if ch >= 2:
                emit_epilogue(ch - 2)
        emit_phase3(3)
        emit_phase4(3)
        emit_epilogue(2)
        emit_epilogue(3)

    nc.compile()
    return nc


def _sigmoid(v):
    return 1.0 / (1.0 + np.exp(-v))


def _make_in_maps(inputs):
    F8NP = mybir.dt.np(F8)
    x = np.asarray(inputs["x"], np.float32)
    w = np.asarray(inputs["w"], np.float64)
    delay = np.asarray(inputs["delay"], np.float64)
    b = np.asarray(inputs["b"], np.float64)
    gate_W = np.asarray(inputs["gate_W"], np.float64)
    gate_b = np.asarray(inputs["gate_b"], np.float64)
    na = np.asarray(inputs["neuron_avg"], np.float64)
    cw1 = np.asarray(inputs["conn_W1"], np.float64)
    cb1 = np.asarray(inputs["conn_b1"], np.float64)
    cw2 = np.asarray(inputs["conn_W2"], np.float64)
    cb2 = np.asarray(inputs["conn_b2"], np.float64)
    mask = np.asarray(inputs["mask"], np.float64)
    actw = np.asarray(inputs["act_w"], np.float64)
    read_W = np.asarray(inputs["read_W"], np.float32)
    read_b = np.asarray(inputs["read_b"], np.float32)
    mem = np.asarray(inputs["memory"], np.float32)

    # connectivity MLP (batch-independent, one row)
    h = np.maximum(na[None, :] @ cw1 + cb1, 0.0)
    conn = (_sigmoid(h @ cw2 + cb2)[0] * mask)            # [U]
    # activation-blend polynomial coefs, prescaled for fp8 blend
    e = np.exp(actw - actw.max())
    wts = e / e.sum()
    coefs = (wts @ _CMAT) * SC_B                          # [5]
    coefs_bc = np.ascontiguousarray(
        np.broadcast_to(coefs.astype(np.float32), (128, 5)))

    xt8 = np.ascontiguousarray(x.T).astype(F8NP)          # [D, N]
    wmod = w * _sigmoid(delay)                            # [D, U, NB]
    wmod = wmod * conn[None, :, None] * SC_W

    in_maps = []
    for k in range(KC):
        us, ue = k * US, (k + 1) * US
        ms, me = k * MS, (k + 1) * MS
        wd8 = np.concatenate(
            [wmod[:, us:ue, :].transpose(0, 2, 1).reshape(D, UBF),
             gate_W * SC_W], axis=1).astype(F8NP)
        bias_row = np.concatenate(
            [(b[us:ue] * conn[us:ue, None]).T.reshape(-1), gate_b]) * SC_W
        mem_aug = np.concatenate(
            [mem[ms:me], np.ones((MS, 1), np.float32)], axis=1)
        in_maps.append({
            "xt": xt8,
            "wd": np.ascontiguousarray(wd8),
            "bias": bias_row.astype(mybir.dt.np(BF16)),
            "coefs": coefs_bc,
            "rw": np.ascontiguousarray(
                (read_W[:, ms:me] * np.float32(SC_W)).astype(F8NP)),
            "rb": np.ascontiguousarray(read_b[ms:me]),
            "mem": np.ascontiguousarray(mem_aug.astype(np.float16)),
        })
    return in_maps


def kernel(**inputs) -> np.ndarray:
    in_maps = _make_in_maps(inputs)
    has_bias = any(np.any(m["bias"] != 0) for m in in_maps)
    key = ("nc", has_bias)
    if key not in _cache:
        _cache[key] = _build(has_bias)
        _cache["nc"] = _cache[key]
    nc = _cache[key]
    res = run_bass_kernel_spmd(nc, in_maps, core_ids=list(range(KC)))
    out = np.empty((N, MD), np.float32)
    for k in range(KC):
        yk = res.results[k]["y"]
        for j in range(4):
            for sj in range(4):
                src = j * 64 + sj * 16
                dst = j * 512 + sj * 128 + k * 16
                out[dst:dst + 16] = yk[src:src + 16]
    return out
